# revision 22
# baseline (speedup 1.0000x reference)
"""Antisymmetric-RNN (ASNN) Trainium2 kernel.

Reference computation (per batch row b):
    A  = W - W^T
    u_t = x_t @ V + b
    s_{t+1} = s_t + EPS * tanh(s_t @ A - GAMMA * s_t + u_t)
    outputs[:, t, :] = s_{t+1};  final_state = s_T

Device mapping (8 NeuronCores, data-parallel over batch, 64 rows each):
  * transposed state layout S = s^T  [W=128 partitions, B=64 free]
  * rescaled state R = S / EPS, Aeps = EPS*(W - W^T - GAMMA*I)  so that
        Q_t   = Aeps^T R_t + U_t           (tanh argument, pre-bias)
        T_t   = tanh(Q_t + b)              (ACT, bias per partition)
        R_t+1 = R_t + T_t                  (DVE, fp32)
    and  Aeps^T R_{t+1} = Aeps^T R_t + Aeps^T T_t, so the PSUM column for
    step t+1 is built by accumulating matmuls with the same stationary
    matrix; the only serial dependency is ACT(tanh) -> PE(matmul) -> ACT.
  * fp32 matmuls on TRN2 cost ~408ns each (2-pass + weight reload); fp16
    matmuls cost ~36ns.  All matmuls therefore run in fp16 with hi/lo
    split-precision operands (A = Ah + Al, V = Vh + Vl, x = xh + xl,
    R = Rh + Rl, each term fp16), keeping ~2^-22 effective operand
    precision.  The tanh feeding the PE is a separate fp16-output ACT op;
    a second fp32 tanh (off the critical path) feeds the exact state
    update, so the state/output path stays fp32-grade.
  * one PSUM bank per timestep column, 8 banks rotating; per column:
    proj (3 fp16 mms, U_c = V^T x_c^T), mm_a (3 fp16 mms, Aeps^T R_{c-1}),
    mm_b (2 fp16 mms, Aeps^T T_{c-1}, closes the bank) -> ACT reads.
  * outputs returned as R_{t+1} tiles [w, t, b]; host scales by EPS and
    transposes back to [b, t, w].
"""

import numpy as np

EPS = 0.01
GAMMA = 0.01
B, T, IND, WIDTH = 512, 512, 64, 128
NCORES = 8
BS = B // NCORES          # 64 batch rows per core
NCHUNK = 8                # timesteps per x-load chunk
NCHUNKS = T // NCHUNK     # 64
RING = 64                 # R ring slots (SBUF)
OUT_HALF = 32             # DMA out every 32 steps

_CACHE = {}


def _split_multiwaits(nc):
    """walrus in this container accepts only 1 sync-wait per instruction
    (2 for EventSemaphore); Tile emits several.  Spill extra waits onto
    NOPs inserted just before the instruction on the same engine queue."""
    import concourse.mybir as mybir

    nsplit = 0
    for fn in nc.m.functions:
        for bb in fn.blocks:
            insts = bb.instructions
            if not any(
                i.sync_info is not None and len(i.sync_info.on_wait) > 1
                for i in insts
            ):
                continue
            new = []
            for inst in insts:
                si = inst.sync_info
                cap = 2 if isinstance(inst, mybir.InstEventSemaphore) else 1
                if si is not None and len(si.on_wait) > cap:
                    waits = list(si.on_wait)
                    extra, keep = waits[:-cap], waits[-cap:]
                    for i, w in enumerate(extra):
                        nop = mybir.InstNoOp(
                            name=f"{inst.name}_wsp{i}", engine=inst.engine)
                        nop.sync_info = mybir.SyncInfo(on_wait=[w], on_update=[])
                        new.append(nop)
                        nsplit += 1
                    inst.sync_info = mybir.SyncInfo(
                        on_wait=keep, on_update=list(si.on_update))
                new.append(inst)
            bb.instructions = new
    return nsplit


def _build(nt=T, split=True, loop_m=None, ablate=()):
    import concourse.bass as bass
    import concourse.mybir as mybir
    from concourse.tile import TileContext

    f32 = mybir.dt.float32
    f16 = mybir.dt.float16
    TANH = mybir.ActivationFunctionType.Tanh
    SUB = mybir.AluOpType.subtract
    nchunks = nt // NCHUNK

    nc = bass.Bass("TRN2", target_bir_lowering=False, debug=False,
                   enable_asserts=False)
    xh_d = nc.dram_tensor("xh", [IND, nt * BS], f16, kind="ExternalInput").ap()
    xl_d = nc.dram_tensor("xl", [IND, nt * BS], f16, kind="ExternalInput").ap()
    r0_d = nc.dram_tensor("r0", [WIDTH, BS], f32, kind="ExternalInput").ap()
    r0h_d = nc.dram_tensor("r0h", [WIDTH, BS], f16, kind="ExternalInput").ap()
    r0l_d = nc.dram_tensor("r0l", [WIDTH, BS], f16, kind="ExternalInput").ap()
    ah_d = nc.dram_tensor("ah", [WIDTH, WIDTH], f16, kind="ExternalInput").ap()
    al_d = nc.dram_tensor("al", [WIDTH, WIDTH], f16, kind="ExternalInput").ap()
    vh_d = nc.dram_tensor("vh", [IND, WIDTH], f16, kind="ExternalInput").ap()
    vl_d = nc.dram_tensor("vl", [IND, WIDTH], f16, kind="ExternalInput").ap()
    b_d = nc.dram_tensor("bvec", [WIDTH, 1], f32, kind="ExternalInput").ap()
    outT = nc.dram_tensor("outT", [WIDTH, nt * BS], f32,
                          kind="ExternalOutput").ap()

    with TileContext(nc) as tc:
        with (
            tc.tile_pool(name="consts", bufs=1) as consts,
            tc.tile_pool(name="xin", bufs=1) as xin,
            tc.tile_pool(name="tt", bufs=4) as ttp,
            tc.tile_pool(name="rhl", bufs=4) as rhl,
            tc.tile_pool(name="ring", bufs=1) as ringp,
            tc.tile_pool(name="q", bufs=8, space="PSUM") as qp,
        ):
            ah = consts.tile([WIDTH, WIDTH], f16)
            nc.sync.dma_start(ah[:], ah_d)
            al = consts.tile([WIDTH, WIDTH], f16)
            nc.sync.dma_start(al[:], al_d)
            vh = consts.tile([IND, WIDTH], f16)
            nc.sync.dma_start(vh[:], vh_d)
            vl = consts.tile([IND, WIDTH], f16)
            nc.sync.dma_start(vl[:], vl_d)
            b_sb = consts.tile([WIDTH, 1], f32)
            nc.sync.dma_start(b_sb[:], b_d)
            r0_sb = consts.tile([WIDTH, BS], f32)
            nc.sync.dma_start(r0_sb[:], r0_d)
            r0h_sb = consts.tile([WIDTH, BS], f16)
            nc.sync.dma_start(r0h_sb[:], r0h_d)
            r0l_sb = consts.tile([WIDTH, BS], f16)
            nc.sync.dma_start(r0l_sb[:], r0l_d)

            ring = ringp.tile([WIDTH, RING * BS], f32)

            qtiles = [None] * nt

            def slot(t):
                s = t % RING
                return ring[:, s * BS:(s + 1) * BS]

            # whole x resident in SBUF (8 MiB/core fp16 split), loaded in
            # XPIECE-step pieces so early projections never wait on the tail
            XPIECE = 32
            xh_all = xin.tile([IND, nt * BS], f16, name="xh_all")
            xl_all = xin.tile([IND, nt * BS], f16, name="xl_all")

            def load_x(p):
                lo, hi = p * BS, (p + XPIECE) * BS
                nc.sync.dma_start(xh_all[:, lo:hi], xh_d[:, lo:hi])
                nc.sync.dma_start(xl_all[:, lo:hi], xl_d[:, lo:hi])

            def proj(c):
                # one PSUM bank per timestep column: U_c = V^T x_c^T,
                # split-precision (drop Vl^T xl ~ 2^-24).
                sl = slice(c * BS, (c + 1) * BS)
                qtiles[c] = qp.tile([WIDTH, BS], f32, tag="q", name=f"qc{c}")
                q = qtiles[c][:, :]
                nc.tensor.matmul(q, lhsT=vh[:], rhs=xh_all[:, sl],
                                 start=True, stop=False)
                nc.tensor.matmul(q, lhsT=vh[:], rhs=xl_all[:, sl],
                                 start=False, stop=False)
                nc.tensor.matmul(q, lhsT=vl[:], rhs=xh_all[:, sl],
                                 start=False, stop=False)

            def mm_a(c, rh, rl):
                # col c += Aeps^T R_{c-3}   (drop Al^T Rl ~ 2^-24)
                q = qtiles[c][:, :]
                nc.tensor.matmul(q, lhsT=ah[:], rhs=rh[:], start=False,
                                 stop=False)
                nc.tensor.matmul(q, lhsT=ah[:], rhs=rl[:], start=False,
                                 stop=False)
                nc.tensor.matmul(q, lhsT=al[:], rhs=rh[:], start=False,
                                 stop=False)

            def mm_t(c, t16, stop=False):
                # col c += Aeps^T T_tau (split);  stop closes the bank
                q = qtiles[c][:, :]
                nc.tensor.matmul(q, lhsT=ah[:], rhs=t16[:], start=False,
                                 stop=False)
                nc.tensor.matmul(q, lhsT=al[:], rhs=t16[:], start=False,
                                 stop=stop)

            def emit_scan():
                # prologue: first x piece in flight, U cols 0..5 in PSUM.
                # column c is assembled as
                #   U_c + Aeps^T R_{c-3} + sum_{j=1..3} Aeps^T T_{c-j}
                # (indices clamped at 0 for the first columns), so the fp32
                # state R only has to reach the PE three rounds after its
                # tanh — the DVE/GPSIMD hi/lo split runs well off the
                # critical path.
                load_x(0)
                for c in range(6):
                    proj(c)
                for c in range(4):
                    mm_a(c, r0h_sb, r0l_sb)
                # close col 0's bank (no T terms in col 0)
                nc.tensor.matmul(qtiles[0][:, :], lhsT=al[:], rhs=r0l_sb[:],
                                 start=False, stop=True)

                prev_r = r0_sb[:, :]
                for t in range(nt):
                    # critical chain at high priority: fp16 tanh of column t
                    # then col t+1 += Aeps^T T_t (closes that bank)
                    with tc.high_priority():
                        t16 = ttp.tile([WIDTH, BS], f16, tag="t16")
                        nc.scalar.activation(t16[:], qtiles[t][:, :], TANH,
                                             bias=b_sb[:, 0:1])
                        if t + 1 < nt:
                            mm_t(t + 1, t16, stop=True)
                    if "nomm23" not in ablate:
                        if t + 2 < nt:
                            mm_t(t + 2, t16)
                        if t + 3 < nt:
                            mm_t(t + 3, t16)

                    if "nostate" not in ablate:
                        # off-critical: exact fp32 tanh for the state update
                        t32 = ttp.tile([WIDTH, BS], f32, tag="t32")
                        nc.scalar.activation(t32[:], qtiles[t][:, :], TANH,
                                             bias=b_sb[:, 0:1])

                        # state update: R_{t+1} = R_t + T32 (fp32, exact)
                        r_new = slot(t)
                        nc.vector.tensor_add(out=r_new, in0=prev_r, in1=t32[:])

                        # split R_{t+1} for the fp16 matmul path (rh on DVE
                        # via fused add, rl on GPSIMD to keep DVE in budget)
                        if t + 4 < nt and "nomma" not in ablate:
                            rh = rhl.tile([WIDTH, BS], f16, tag="rh")
                            nc.vector.scalar_tensor_tensor(
                                out=rh[:], in0=prev_r, scalar=1.0, in1=t32[:],
                                op0=mybir.AluOpType.mult,
                                op1=mybir.AluOpType.add)
                            rl = rhl.tile([WIDTH, BS], f16, tag="rl")
                            if "nogps" not in ablate:
                                nc.gpsimd.tensor_tensor(out=rl[:], in0=r_new,
                                                        in1=rh[:], op=SUB)
                            else:
                                nc.vector.tensor_tensor(out=rl[:], in0=r_new,
                                                        in1=rh[:], op=SUB)
                            mm_a(t + 4, rh, rl)
                    else:
                        r_new = prev_r

                    # project column t+6 into the bank ACT(t-2) just freed
                    if t + 6 < nt:
                        proj(t + 6)

                    if t % XPIECE == 1 and t - 1 + XPIECE < nt:
                        load_x(t - 1 + XPIECE)

                    if t % OUT_HALF == OUT_HALF - 1 and "nodma" not in ablate \
                            and "nostate" not in ablate:
                        t0 = t - OUT_HALF + 1
                        s0 = t0 % RING
                        nc.sync.dma_start(
                            outT[:, t0 * BS:(t + 1) * BS],
                            ring[:, s0 * BS:(s0 + OUT_HALF) * BS])

                    prev_r = r_new

            if loop_m is None:
                emit_scan()
            else:
                with tc.For_i(0, loop_m, 1):
                    emit_scan()

    if split:
        _split_multiwaits(nc)
    return nc


def _get_nc():
    if "nc" not in _CACHE:
        _CACHE["nc"] = _build()
    return _CACHE["nc"]


def _split16(a):
    hi = a.astype(np.float16)
    lo = (a - hi.astype(np.float32)).astype(np.float16)
    return hi, lo


def _host_prep(x, init_state, W, V, b):
    W = np.asarray(W, np.float32)
    aeps = (EPS * ((W - W.T) - GAMMA * np.eye(WIDTH, dtype=np.float32))
            ).astype(np.float32)
    ah, al = _split16(aeps)
    vh, vl = _split16(np.ascontiguousarray(np.asarray(V, np.float32)))
    bvec = np.asarray(b, np.float32).reshape(WIDTH, 1).copy()
    x = np.asarray(x, np.float32)
    s0 = np.asarray(init_state, np.float32)[0]          # [B, W]
    r0_full = np.ascontiguousarray((s0 / np.float32(EPS)).T)  # [W, B]
    in_maps = []
    for c in range(NCORES):
        xc = x[c * BS:(c + 1) * BS]                      # [BS, T, IND]
        xTc = np.ascontiguousarray(xc.transpose(2, 1, 0)).reshape(IND, T * BS)
        xh, xl = _split16(xTc)
        r0 = np.ascontiguousarray(r0_full[:, c * BS:(c + 1) * BS])
        r0h, r0l = _split16(r0)
        in_maps.append({
            "xh": xh, "xl": xl,
            "r0": r0, "r0h": r0h, "r0l": r0l,
            "ah": ah, "al": al, "vh": vh, "vl": vl,
            "bvec": bvec,
        })
    return in_maps


def _gather(results):
    outputs = np.empty((B, T, WIDTH), np.float32)
    eps = np.float32(EPS)
    for c in range(NCORES):
        oc = results[c]["outT"].reshape(WIDTH, T, BS)    # [w, t, b] = R_{t+1}
        outputs[c * BS:(c + 1) * BS] = oc.transpose(2, 1, 0) * eps
    final_state = outputs[:, -1, :].copy()
    return outputs, final_state


def _run(inputs, trace=False, **kw):
    from concourse.bass_utils import run_bass_kernel_spmd
    nc = _get_nc()
    in_maps = _host_prep(**inputs)
    res = run_bass_kernel_spmd(nc, in_maps, core_ids=list(range(NCORES)),
                               trace=trace, **kw)
    outputs, final_state = _gather(res.results)
    return outputs, final_state, res


def kernel(x, init_state, W, V, b):
    outputs, final_state, _ = _run(
        dict(x=x, init_state=init_state, W=W, V=V, b=b))
    return outputs, final_state
